# Initial kernel scaffold
#
"""CLIP cross-attention (pre-LN QKV + softmax attention + bottleneck adapter)
on 8 Trainium2 NeuronCores, batch-data-parallel (1 batch element per core).

Per-core dataflow (S=1024 tokens, H=1024, 16 heads x 64):
  LN in natural layout (bn_stats/apply on DVE, gamma/beta folded into the
  projection weights on the host); the LN apply writes bf16 staging rows
  into qT/kT space (empty during LN) so the PE transposes run in bf16 at
  1 cyc/row, evicted to [H, S] bf16 activations on ScalarE.  QKV
  projections run in bf16.  Scores are computed transposed per head-PAIR
  (S^T = K^T.T @ Q^T): the two heads of a pair live in partitions 0:64 /
  64:128 of kT/qT, so their score matmuls are issued interleaved and run
  concurrently in disjoint PE row-groups (2x effective rate at K=64).
  Exp on ScalarE in [128,1024] batches, no max-subtraction (|scores| <=
  ~9); psum double-buffering gives a 2-exp gate depth so the stream never
  serializes on the quad->exp chain.  The softmax denominator falls out
  of the PV matmul via an appended ones column in V; PV emits natural-
  layout attention rows, normalized via reciprocal+scale on DVE into
  bf16 attn ALIASED into qT storage (qT chunk j is dead once pair j is
  scored; pair j attention lands exactly there).  Adapter: D^T = Wd.T @
  attn^T (bf16 PE transposes), tanh-gelu, U = G^T.T @ [Wu;bu], residual
  add, store.

Schedule: the ScalarE exp stream (~147us) is the global pacer: ctx-LN ->
hs-LN -> V -> per-pair {K/Q pieces for the next pair popped at every odd
slot (4/pair - fewer leaves stale staging in qT), PV of the previous
pair, paired score quads + exps}.  Weight and constant DMAs ride the
GPSIMD SWDGE queue so slot-waiting loads never block the SP/ACT queues.
Measured ~260us HW (from 356805ns baseline; 270018ns before the
in-stream attn transposes + quads-first slot order, then -5 to -10us
per rep in same-process A/B), rel_err 4.8e-3.  attn^T lives in dead kT
chunks; pairs 0-5 transpose during the stream, 6-7 in the adapter tail.
"""

import numpy as np
import ml_dtypes

import concourse.bass as bass
import concourse.tile as tile
from concourse import bacc, mybir
from concourse.bass_utils import run_bass_kernel_spmd
from concourse.masks import make_identity
from contextlib import ExitStack

F32 = mybir.dt.float32
F32R = mybir.dt.float32r
BF16 = mybir.dt.bfloat16
AF = mybir.ActivationFunctionType
ALU = mybir.AluOpType

S = 1024
H = 1024
NH = 16
HD = 64
P = 128
NCORES = 8
EPS = 1e-5


def build_program(reps=1):
    nc = bacc.Bacc("TRN2", target_bir_lowering=False, debug=False,
                   num_devices=NCORES)

    hs = nc.dram_tensor("hs", [S, H], F32R, kind="ExternalInput")
    cx = nc.dram_tensor("cx", [S, H], F32R, kind="ExternalInput")
    wq = nc.dram_tensor("wq", [H, H], BF16, kind="ExternalInput")
    wk = nc.dram_tensor("wk", [H, H], BF16, kind="ExternalInput")
    wv = nc.dram_tensor("wv", [H, H], BF16, kind="ExternalInput")
    bq = nc.dram_tensor("bq", [P, 8], F32, kind="ExternalInput")
    bk = nc.dram_tensor("bk", [P, 8], F32, kind="ExternalInput")
    bv = nc.dram_tensor("bv", [1, H], BF16, kind="ExternalInput")
    wd = nc.dram_tensor("wd", [H, HD], BF16, kind="ExternalInput")
    bd = nc.dram_tensor("bd", [HD, 1], F32, kind="ExternalInput")
    wub = nc.dram_tensor("wub", [HD + 1, H], BF16, kind="ExternalInput")
    out = nc.dram_tensor("out", [S, H], F32, kind="ExternalOutput")

    with tile.TileContext(nc) as tc, ExitStack() as ctx:
        pc = ctx.enter_context(tc.tile_pool(name="const", bufs=1))
        pbig = ctx.enter_context(tc.tile_pool(name="big", bufs=2))
        ppt = ctx.enter_context(tc.tile_pool(name="ptile", bufs=4))
        pw = ctx.enter_context(tc.tile_pool(name="w", bufs=18))
        pq = ctx.enter_context(tc.tile_pool(name="q", bufs=1))
        pk = ctx.enter_context(tc.tile_pool(name="k", bufs=1))
        pv = ctx.enter_context(tc.tile_pool(name="v", bufs=1))
        pxl = ctx.enter_context(tc.tile_pool(name="xl", bufs=3))
        pstat = ctx.enter_context(tc.tile_pool(name="stat", bufs=2))
        pout = ctx.enter_context(tc.tile_pool(name="outp", bufs=3))
        pg = ctx.enter_context(tc.tile_pool(name="g", bufs=1))
        pps_t = ctx.enter_context(tc.tile_pool(name="pst", bufs=2, space="PSUM"))
        pps_m = ctx.enter_context(tc.tile_pool(name="psm", bufs=2, space="PSUM"))
        pps_o = ctx.enter_context(tc.tile_pool(name="pso", bufs=2, space="PSUM"))

        # constants / small inputs (GPSIMD so SP starts on ctx tiles at t=0)
        idb = pc.tile([P, P], BF16)
        make_identity(nc, idb[:])
        bq_sb = pc.tile([P, 8], F32)
        nc.gpsimd.dma_start(bq_sb[:], bq[:])
        bk_sb = pc.tile([P, 8], F32)
        nc.gpsimd.dma_start(bk_sb[:], bk[:])
        bv_sb = pc.tile([P, H], BF16)
        nc.gpsimd.dma_start(bv_sb[:], bv[:].partition_broadcast(P)[:, 0, :])
        wd_sb = pc.tile([P, 8, HD], BF16)
        nc.gpsimd.dma_start(wd_sb[:], wd[:].rearrange("(c p) a -> p c a", p=P))
        bd_sb = pc.tile([HD, 1], F32)
        nc.gpsimd.dma_start(bd_sb[:], bd[:])
        wub_sb = pc.tile([HD + 1, H], BF16)
        nc.gpsimd.dma_start(wub_sb[:], wub[:])
        eps_sb = pc.tile([P, 1], F32)
        nc.vector.memset(eps_sb[:], EPS)

        vt = pv.tile([P, 8, NH, HD + 1], BF16, tag="V")
        nc.vector.memset(vt[:, :, :, HD:HD + 1], 1.0)
        ones1 = pc.tile([1, P], BF16)
        nc.vector.memset(ones1[:], 1.0)
        gt = pg.tile([HD + 1, H], BF16, tag="gt")
        nc.vector.memset(gt[HD:HD + 1, :], 1.0)

        qT = pq.tile([P, 8, S], BF16, tag="qT")
        kT = pk.tile([P, 8, S], BF16, tag="kT")
        # attn (natural, bf16) aliases qT's storage: element (s, h) with
        # s = (s_lo, m), h = (pair j, dd) lives at qT[s_lo, j, m*128 + dd].
        # qT chunk j is dead (pair j's scores issued) before pair j's PV
        # writes land there.
        attn = qT[:].rearrange("p j (m dd) -> p j m dd", dd=P)

        loop_ctx = ExitStack()
        if reps > 1:
            hints = (nc.tensor.engine, nc.vector.engine, nc.scalar.engine,
                     nc.sync.engine)
            loop_ctx.enter_context(tc.For_i(0, reps, 1, hint_engines=hints))
        ctx.enter_context(loop_ctx)

        # warm the Sqrt ACT table while the first DMAs are in flight
        warm = pc.tile([P, 1], F32)
        nc.scalar.activation(warm[:], eps_sb[:], AF.Sqrt, bias=eps_sb[:])

        def load_w(wdram):
            tiles = []
            for kk in range(8):
                wt = pw.tile([P, H], BF16, tag="wc")
                nc.gpsimd.dma_start(wt[:], wdram[kk * P:(kk + 1) * P, :])
                tiles.append(wt)
            return tiles

        def ln_transpose(xdram, dstT, stageT):
            # LN in natural layout: stats on DVE, apply on DVE into a bf16
            # staging row of stageT (qT/kT space, empty during LN); then
            # bf16 PE-transposes (1 cyc/row), evicted on ScalarE.
            for m in range(8):
                xt = pxl.tile([P, H], F32R, tag="xl")
                nc.sync.dma_start(xt[:], xdram[m * P:(m + 1) * P, :])
                x32 = xt[:].bitcast(F32)
                st = pstat.tile([P, 2, 6], F32, tag="st")
                nc.vector.bn_stats(st[:, 0, :], x32[:, 0:512])
                nc.vector.bn_stats(st[:, 1, :], x32[:, 512:1024])
                mv = pstat.tile([P, 2], F32, tag="mv")
                nc.vector.bn_aggr(mv[:], st[:])
                sd = pstat.tile([P, 1], F32, tag="sd")
                nc.scalar.activation(sd[:], mv[:, 1:2], AF.Sqrt, bias=eps_sb[:])
                rstd = pstat.tile([P, 1], F32, tag="rs")
                nc.vector.reciprocal(rstd[:], sd[:])
                xb = stageT[:, m, :]
                nc.vector.tensor_scalar(xb, x32, mv[:, 0:1], rstd[:],
                                        ALU.subtract, ALU.mult)
                xbb = xb.rearrange("p (b c) -> p b c", c=P)
                for j in range(2):
                    pt32 = pps_t.tile([P, 512], F32, tag="pt")
                    pt = pt32[:].bitcast(BF16)[:, 0:512]
                    for jj in range(4):
                        nc.tensor.transpose(pt[:, jj * P:(jj + 1) * P],
                                            xbb[:, j * 4 + jj, :], idb[:])
                    nc.scalar.copy(
                        dstT[:, j * 4:(j + 1) * 4, m * P:(m + 1) * P],
                        pt.rearrange("p (jj c) -> p jj c", c=P))

        def proj_piece(wtiles, srcT, dstT, bias_sb, m8, n2):
            # one 512-col half of dstT[:, m8, :] = (W.T @ src^T) + bias
            # (bf16, DVE evict - ScalarE must stay free for the exp stream).
            # Uses the transpose-pool psum tag, idle during attention, so a
            # piece never competes with the exp-gated score psums.
            pm = pps_t.tile([P, 512], F32, tag="pt")
            for kk in range(8):
                nc.tensor.matmul(
                    pm[:], wtiles[kk][:, m8 * P:(m8 + 1) * P],
                    srcT[:, kk, n2 * 512:(n2 + 1) * 512],
                    start=(kk == 0), stop=(kk == 7))
            nc.vector.tensor_scalar(
                dstT[:, m8, n2 * 512:(n2 + 1) * 512], pm[:],
                bias_sb[:, m8:m8 + 1], None, ALU.add)

        # ---- LN both inputs (wv first: V-proj runs before K/Q need theirs)
        ctxT = pbig.tile([P, 8, S], BF16, tag="big")
        wv_t = load_w(wv)
        wk_t = load_w(wk)
        wq_t = load_w(wq)
        ln_transpose(cx, ctxT, qT)
        hsT = pbig.tile([P, 8, S], BF16, tag="big")
        ln_transpose(hs, hsT, kT)

        # ---- V projection.  bv is injected into the PSUM accumulation via
        # a K=1 ones-row matmul so the evict is a plain ScalarE copy (a DVE
        # evict would queue behind the hs-LN stats and stall V on psum slots)
        for c in range(8):
            pm = pps_m.tile([P, 1024], F32, tag="pm")
            for n2 in range(2):
                nc.tensor.matmul(pm[:, n2 * 512:(n2 + 1) * 512], ones1[:],
                                 bv_sb[0:1, n2 * 512:(n2 + 1) * 512],
                                 start=True, stop=False)
                for kk in range(8):
                    nc.tensor.matmul(
                        pm[:, n2 * 512:(n2 + 1) * 512],
                        ctxT[:, kk, c * P:(c + 1) * P],
                        wv_t[kk][:, n2 * 512:(n2 + 1) * 512],
                        start=False, stop=(kk == 7))
            nc.scalar.copy(vt[:, c, :, 0:HD],
                           pm[:].rearrange("p (h c) -> p h c", c=HD))

        # ---- K/Q projections interleaved with attention.
        pT_tiles = {}

        def scores_head(h, c):
            # one head's [128, 1024] score chunk: 2 matmuls (the pair's
            # other head runs in the opposite PE row-group, so adjacent
            # A/B matmuls overlap in the array), one Exp evict.
            r0 = (h % 2) * HD
            hc = h // 2
            pm = pps_m.tile([P, 1024], F32, tag="pm", name=f"pm{h}_{c}")
            for n2 in range(2):
                nc.tensor.matmul(
                    pm[:, n2 * 512:(n2 + 1) * 512],
                    kT[r0:r0 + HD, hc, c * P:(c + 1) * P],
                    qT[r0:r0 + HD, hc, n2 * 512:(n2 + 1) * 512],
                    start=True, stop=True)
            nc.scalar.activation(pT_tiles[h][:, c, :], pm[:], AF.Exp,
                                 scale=0.125)

        def pv_m2(h, m):
            # two token-chunks (m, m+1) share one PV accumulator tile and
            # one batched reciprocal: half the DVE instruction count
            pT = pT_tiles[h]
            po = pps_o.tile([P, 2, HD + 1], F32, tag="po")
            for mi in range(2):
                for c in range(8):
                    nc.tensor.matmul(
                        po[:, mi, :],
                        pT[:, c, (m + mi) * P:(m + mi + 1) * P],
                        vt[:, c, h, :],
                        start=(c == 0), stop=(c == 7))
            rs = pstat.tile([P, 2], F32, tag="rs2")
            nc.vector.reciprocal(rs[:], po[:, :, HD])
            for mi in range(2):
                nc.vector.tensor_scalar(
                    attn[:, h // 2, m + mi, (h % 2) * HD:(h % 2) * HD + HD],
                    po[:, mi, 0:HD], rs[:, mi:mi + 1], None, ALU.mult)

        # attn^T aliases kT: chunk j of kT is dead once pair j's score
        # quads issue, and pair j's attn rows are transposed two pairs
        # later (pairs 0-5 in-stream, pairs 6-7 in the adapter tail).
        attn_T = kT[:]

        def tp_attn(j, m, pt):
            # one [128,128] transpose of pair j's attn block for token
            # chunk m into a held psum quad (evicted every 4th)
            nc.tensor.transpose(pt[:, (m % 4) * P:(m % 4 + 1) * P],
                                attn[:, j, m, :], idb[:])

        def tp_evict(j, m4, pt):
            nc.vector.tensor_copy(
                attn_T[:, j, m4 * 512:(m4 + 1) * 512].rearrange(
                    "p (mm c) -> p mm c", c=P),
                pt.rearrange("p (mm c) -> p mm c", c=P))

        # prologue: first K/Q chunk
        for n2 in range(2):
            proj_piece(wk_t, ctxT, kT, bk_sb, 0, n2)
        for n2 in range(2):
            proj_piece(wq_t, hsT, qT, bq_sb, 0, n2)
        for m8 in range(8):
            # next iteration's K/Q chunks, spread through the score stream
            # so ScalarE's exp pipeline is never starved by them
            pieces = []
            if m8 + 1 < 8:
                for wt, st_, dt_, bs in ((wk_t, ctxT, kT, bk_sb),
                                         (wq_t, hsT, qT, bq_sb)):
                    for n2 in range(2):
                        pieces.append((wt, st_, dt_, bs, m8 + 1, n2))
            hA, hB = 2 * m8, 2 * m8 + 1
            pT_tiles[hA] = ppt.tile([P, 8, S], BF16, tag="pT",
                                    name=f"pT{hA}")
            pT_tiles[hB] = ppt.tile([P, 8, S], BF16, tag="pT",
                                    name=f"pT{hB}")
            tpq = None
            for c in range(8):
                # score quads first: their psum WAR stalls at the queue
                # head, but the exp then fires right after the quad
                # instead of trailing the whole slot's work; pv, attn
                # transposes and proj pieces fill the exp window behind
                scores_head(hA, c)
                scores_head(hB, c)
                if m8 > 0:
                    pv_m2(hA - 2 if c % 2 == 0 else hB - 2, (c // 2) * 2)
                if m8 >= 2:
                    if c % 4 == 0:
                        tpq = pps_t.tile([P, 512], F32, tag="pt")
                        tpq = tpq[:].bitcast(BF16)[:, 0:512]
                    tp_attn(m8 - 2, c, tpq)
                    if c % 4 == 3:
                        tp_evict(m8 - 2, c // 4, tpq)
                if c % 2 == 1 and pieces:
                    proj_piece(*pieces.pop(0))

        # ---- adapter + residual (pairs 6-7: PV and attn^T transposes fold
        # into the m loop; pairs 0-5 were transposed during the stream).
        # The second token-half's PV/transposes interleave into the first
        # half's up-projection loop so the two n2 chains pipeline.
        def adapter_pre(n2):
            for mi in range(0, 4, 2):
                pv_m2(NH - 2, n2 * 4 + mi)
                pv_m2(NH - 1, n2 * 4 + mi)
            for j67 in (6, 7):
                pt32 = pps_t.tile([P, 512], F32, tag="pt")
                ptb = pt32[:].bitcast(BF16)[:, 0:512]
                for mi in range(4):
                    m = n2 * 4 + mi
                    nc.tensor.transpose(ptb[:, mi * P:(mi + 1) * P],
                                        attn[:, j67, m, :], idb[:])
                nc.scalar.copy(
                    attn_T[:, j67, n2 * 512:(n2 + 1) * 512].rearrange(
                        "p (mm c) -> p mm c", c=P),
                    ptb.rearrange("p (mm c) -> p mm c", c=P))

        def adapter_down(n2):
            pd = pps_m.tile([P, 1024], F32, tag="pm")
            for kk in range(8):
                nc.tensor.matmul(pd[0:HD, 0:512], wd_sb[:, kk, :],
                                 attn_T[:, kk, n2 * 512:(n2 + 1) * 512],
                                 start=(kk == 0), stop=(kk == 7))
            nc.scalar.activation(gt[0:HD, n2 * 512:(n2 + 1) * 512],
                                 pd[0:HD, 0:512], AF.Gelu_apprx_tanh,
                                 bias=bd_sb[:])

        def adapter_up_m(n2, mi):
            m = n2 * 4 + mi
            pu = pps_m.tile([P, 1024], F32, tag="pm")
            for nH in range(2):
                nc.tensor.matmul(pu[:, nH * 512:(nH + 1) * 512],
                                 gt[:, m * P:(m + 1) * P],
                                 wub_sb[:, nH * 512:(nH + 1) * 512],
                                 start=True, stop=True)
                ot = pout.tile([P, 512], F32, tag="out")
                nc.vector.tensor_tensor(
                    ot[:].rearrange("p (j dd) -> p j dd", dd=P),
                    pu[:, nH * 512:(nH + 1) * 512].rearrange(
                        "p (j dd) -> p j dd", dd=P),
                    attn[:, nH * 4:(nH + 1) * 4, m, :],
                    ALU.add)
                nc.sync.dma_start(
                    out[m * P:(m + 1) * P, nH * 512:(nH + 1) * 512],
                    ot[:])

        adapter_pre(0)
        adapter_down(0)
        adapter_pre(1)
        for mi in range(4):
            adapter_up_m(0, mi)
        adapter_down(1)
        for mi in range(4):
            adapter_up_m(1, mi)

    nc.compile()
    return nc


def make_in_maps(hidden_states, context, Wq, bq, Wk, bk, Wv, bv,
                 q_gamma, q_beta, c_gamma, c_beta, Wd, bd, Wu, bu):
    f32 = np.float32
    bf = ml_dtypes.bfloat16
    # fold LN gamma/beta into the projection weights (host-side)
    wq_e = (q_gamma[:, None] * Wq).astype(bf)
    bq_e = (bq + q_beta @ Wq).astype(f32)
    wk_e = (c_gamma[:, None] * Wk).astype(bf)
    bk_e = (bk + c_beta @ Wk).astype(f32)
    wv_e = (c_gamma[:, None] * Wv).astype(bf)
    bv_e = (bv + c_beta @ Wv).astype(f32)

    bq_r = np.ascontiguousarray(bq_e.reshape(8, P).T)   # [P, 8]
    bk_r = np.ascontiguousarray(bk_e.reshape(8, P).T)
    bv_r = bv_e.reshape(1, H).astype(bf)
    wd_b = Wd.astype(bf)
    bd_r = bd.reshape(HD, 1).astype(f32)
    wub = np.vstack([Wu, bu.reshape(1, H)]).astype(bf)

    shared = {
        "wq": np.ascontiguousarray(wq_e), "wk": np.ascontiguousarray(wk_e),
        "wv": np.ascontiguousarray(wv_e),
        "bq": bq_r, "bk": bk_r, "bv": bv_r,
        "wd": wd_b, "bd": bd_r, "wub": wub,
    }
    in_maps = []
    for b_ in range(NCORES):
        m = dict(shared)
        m["hs"] = np.ascontiguousarray(hidden_states[b_]).astype(f32)
        m["cx"] = np.ascontiguousarray(context[b_]).astype(f32)
        in_maps.append(m)
    return in_maps


_CACHE = {}


def get_program(reps=1):
    if reps not in _CACHE:
        _CACHE[reps] = build_program(reps=reps)
    return _CACHE[reps]


def kernel(**inputs):
    nc = get_program()
    in_maps = make_in_maps(**{k: np.asarray(v) for k, v in inputs.items()})
    res = run_bass_kernel_spmd(nc, in_maps, list(range(NCORES)))
    out = np.stack([res.results[c]["out"] for c in range(NCORES)], axis=0)
    return out.astype(np.float32)



# revision 1
# speedup vs baseline: 2.2859x; 2.2859x over previous
"""CLIP cross-attention (pre-LN QKV + softmax attention + bottleneck adapter)
on 8 Trainium2 NeuronCores, batch-data-parallel (1 batch element per core).

Per-core dataflow (S=1024 tokens, H=1024, 16 heads x 64):
  LN in natural layout (bn_stats/apply on DVE, gamma/beta folded into the
  projection weights on the host); the LN apply writes bf16 staging rows
  into qT/kT space (empty during LN) so the PE transposes run in bf16 at
  1 cyc/row, evicted to [H, S] bf16 activations on ScalarE.  QKV
  projections run in bf16.  Scores are computed transposed per head-PAIR
  (S^T = K^T.T @ Q^T): the two heads of a pair live in partitions 0:64 /
  64:128 of kT/qT, so their score matmuls are issued interleaved and run
  concurrently in disjoint PE row-groups (2x effective rate at K=64).
  Exp on ScalarE in [128,1024] batches, no max-subtraction (|scores| <=
  ~9); psum double-buffering gives a 2-exp gate depth so the stream never
  serializes on the quad->exp chain.  The softmax denominator falls out
  of the PV matmul via an appended ones column in V; PV emits natural-
  layout attention rows, normalized via reciprocal+scale on DVE into
  bf16 attn ALIASED into qT storage (qT chunk j is dead once pair j is
  scored; pair j attention lands exactly there).  Adapter: D^T = Wd.T @
  attn^T (bf16 PE transposes), tanh-gelu, U = G^T.T @ [Wu;bu], residual
  add, store.

Schedule: the ScalarE exp stream (~147us) is the global pacer: ctx-LN ->
hs-LN -> V -> per-pair {K/Q pieces for the next pair popped at every odd
slot (4/pair - fewer leaves stale staging in qT), PV of the previous
pair, paired score quads + exps}.  Weight and constant DMAs ride the
GPSIMD SWDGE queue so slot-waiting loads never block the SP/ACT queues.
Measured ~260us HW (from 356805ns baseline; 270018ns before the
in-stream attn transposes + quads-first slot order, then -5 to -10us
per rep in same-process A/B), rel_err 4.8e-3.  attn^T lives in dead kT
chunks; pairs 0-5 transpose during the stream, 6-7 in the adapter tail.
"""

import numpy as np
import ml_dtypes

import concourse.bass as bass
import concourse.tile as tile
from concourse import bacc, mybir
from concourse.bass_utils import run_bass_kernel_spmd
from concourse.masks import make_identity
from contextlib import ExitStack

F32 = mybir.dt.float32
F32R = mybir.dt.float32r
BF16 = mybir.dt.bfloat16
AF = mybir.ActivationFunctionType
ALU = mybir.AluOpType

S = 1024
H = 1024
NH = 16
HD = 64
P = 128
NCORES = 8
EPS = 1e-5


def build_program(reps=1):
    nc = bacc.Bacc("TRN2", target_bir_lowering=False, debug=False,
                   num_devices=NCORES)

    hs = nc.dram_tensor("hs", [S, H], F32R, kind="ExternalInput")
    cx = nc.dram_tensor("cx", [S, H], F32R, kind="ExternalInput")
    wq = nc.dram_tensor("wq", [H, H], BF16, kind="ExternalInput")
    wk = nc.dram_tensor("wk", [H, H], BF16, kind="ExternalInput")
    wv = nc.dram_tensor("wv", [H, H], BF16, kind="ExternalInput")
    bq = nc.dram_tensor("bq", [P, 8], F32, kind="ExternalInput")
    bk = nc.dram_tensor("bk", [P, 8], F32, kind="ExternalInput")
    bv = nc.dram_tensor("bv", [1, H], BF16, kind="ExternalInput")
    wd = nc.dram_tensor("wd", [H, HD], BF16, kind="ExternalInput")
    bd = nc.dram_tensor("bd", [HD, 1], F32, kind="ExternalInput")
    wub = nc.dram_tensor("wub", [HD + 1, H], BF16, kind="ExternalInput")
    out = nc.dram_tensor("out", [S, H], F32, kind="ExternalOutput")

    with tile.TileContext(nc) as tc, ExitStack() as ctx:
        pc = ctx.enter_context(tc.tile_pool(name="const", bufs=1))
        pbig = ctx.enter_context(tc.tile_pool(name="big", bufs=2))
        ppt = ctx.enter_context(tc.tile_pool(name="ptile", bufs=4))
        pw = ctx.enter_context(tc.tile_pool(name="w", bufs=18))
        pq = ctx.enter_context(tc.tile_pool(name="q", bufs=1))
        pk = ctx.enter_context(tc.tile_pool(name="k", bufs=1))
        pv = ctx.enter_context(tc.tile_pool(name="v", bufs=1))
        pxl = ctx.enter_context(tc.tile_pool(name="xl", bufs=3))
        pstat = ctx.enter_context(tc.tile_pool(name="stat", bufs=2))
        pout = ctx.enter_context(tc.tile_pool(name="outp", bufs=3))
        pg = ctx.enter_context(tc.tile_pool(name="g", bufs=1))
        pps_t = ctx.enter_context(tc.tile_pool(name="pst", bufs=2, space="PSUM"))
        pps_m = ctx.enter_context(tc.tile_pool(name="psm", bufs=2, space="PSUM"))
        pps_o = ctx.enter_context(tc.tile_pool(name="pso", bufs=2, space="PSUM"))

        # constants / small inputs (GPSIMD so SP starts on ctx tiles at t=0)
        idb = pc.tile([P, P], BF16)
        make_identity(nc, idb[:])
        bq_sb = pc.tile([P, 8], F32)
        nc.gpsimd.dma_start(bq_sb[:], bq[:])
        bk_sb = pc.tile([P, 8], F32)
        nc.gpsimd.dma_start(bk_sb[:], bk[:])
        bv_sb = pc.tile([P, H], BF16)
        nc.gpsimd.dma_start(bv_sb[:], bv[:].partition_broadcast(P)[:, 0, :])
        wd_sb = pc.tile([P, 8, HD], BF16)
        nc.gpsimd.dma_start(wd_sb[:], wd[:].rearrange("(c p) a -> p c a", p=P))
        bd_sb = pc.tile([HD, 1], F32)
        nc.gpsimd.dma_start(bd_sb[:], bd[:])
        wub_sb = pc.tile([HD + 1, H], BF16)
        nc.gpsimd.dma_start(wub_sb[:], wub[:])
        eps_sb = pc.tile([P, 1], F32)
        nc.vector.memset(eps_sb[:], EPS)

        vt = pv.tile([P, 8, NH, HD + 1], BF16, tag="V")
        nc.vector.memset(vt[:, :, :, HD:HD + 1], 1.0)
        ones1 = pc.tile([1, P], BF16)
        nc.vector.memset(ones1[:], 1.0)
        gt = pg.tile([HD + 1, H], BF16, tag="gt")
        nc.vector.memset(gt[HD:HD + 1, :], 1.0)

        qT = pq.tile([P, 8, S], BF16, tag="qT")
        kT = pk.tile([P, 8, S], BF16, tag="kT")
        # attn (natural, bf16) aliases qT's storage: element (s, h) with
        # s = (s_lo, m), h = (pair j, dd) lives at qT[s_lo, j, m*128 + dd].
        # qT chunk j is dead (pair j's scores issued) before pair j's PV
        # writes land there.
        attn = qT[:].rearrange("p j (m dd) -> p j m dd", dd=P)

        loop_ctx = ExitStack()
        if reps > 1:
            hints = (nc.tensor.engine, nc.vector.engine, nc.scalar.engine,
                     nc.sync.engine)
            loop_ctx.enter_context(tc.For_i(0, reps, 1, hint_engines=hints))
        ctx.enter_context(loop_ctx)

        # warm the Sqrt ACT table while the first DMAs are in flight
        warm = pc.tile([P, 1], F32)
        nc.scalar.activation(warm[:], eps_sb[:], AF.Sqrt, bias=eps_sb[:])

        def load_w(wdram):
            tiles = []
            for kk in range(8):
                wt = pw.tile([P, H], BF16, tag="wc")
                nc.gpsimd.dma_start(wt[:], wdram[kk * P:(kk + 1) * P, :])
                tiles.append(wt)
            return tiles

        def ln_transpose(xdram, dstT, stageT):
            # LN in natural layout: stats on DVE, apply on DVE into a bf16
            # staging row of stageT (qT/kT space, empty during LN); then
            # bf16 PE-transposes (1 cyc/row), evicted on ScalarE.
            for m in range(8):
                xt = pxl.tile([P, H], F32R, tag="xl")
                nc.sync.dma_start(xt[:], xdram[m * P:(m + 1) * P, :])
                x32 = xt[:].bitcast(F32)
                st = pstat.tile([P, 2, 6], F32, tag="st")
                nc.vector.bn_stats(st[:, 0, :], x32[:, 0:512])
                nc.vector.bn_stats(st[:, 1, :], x32[:, 512:1024])
                mv = pstat.tile([P, 2], F32, tag="mv")
                nc.vector.bn_aggr(mv[:], st[:])
                sd = pstat.tile([P, 1], F32, tag="sd")
                nc.scalar.activation(sd[:], mv[:, 1:2], AF.Sqrt, bias=eps_sb[:])
                rstd = pstat.tile([P, 1], F32, tag="rs")
                nc.vector.reciprocal(rstd[:], sd[:])
                xb = stageT[:, m, :]
                nc.vector.tensor_scalar(xb, x32, mv[:, 0:1], rstd[:],
                                        ALU.subtract, ALU.mult)
                xbb = xb.rearrange("p (b c) -> p b c", c=P)
                for j in range(2):
                    pt32 = pps_t.tile([P, 512], F32, tag="pt")
                    pt = pt32[:].bitcast(BF16)[:, 0:512]
                    for jj in range(4):
                        nc.tensor.transpose(pt[:, jj * P:(jj + 1) * P],
                                            xbb[:, j * 4 + jj, :], idb[:])
                    nc.scalar.copy(
                        dstT[:, j * 4:(j + 1) * 4, m * P:(m + 1) * P],
                        pt.rearrange("p (jj c) -> p jj c", c=P))

        def proj_piece(wtiles, srcT, dstT, bias_sb, m8, n2):
            # one 512-col half of dstT[:, m8, :] = (W.T @ src^T) + bias
            # (bf16, DVE evict - ScalarE must stay free for the exp stream).
            # Uses the transpose-pool psum tag, idle during attention, so a
            # piece never competes with the exp-gated score psums.
            pm = pps_t.tile([P, 512], F32, tag="pt")
            for kk in range(8):
                nc.tensor.matmul(
                    pm[:], wtiles[kk][:, m8 * P:(m8 + 1) * P],
                    srcT[:, kk, n2 * 512:(n2 + 1) * 512],
                    start=(kk == 0), stop=(kk == 7))
            nc.vector.tensor_scalar(
                dstT[:, m8, n2 * 512:(n2 + 1) * 512], pm[:],
                bias_sb[:, m8:m8 + 1], None, ALU.add)

        # ---- LN both inputs (wv first: V-proj runs before K/Q need theirs)
        ctxT = pbig.tile([P, 8, S], BF16, tag="big")
        wv_t = load_w(wv)
        wk_t = load_w(wk)
        wq_t = load_w(wq)
        ln_transpose(cx, ctxT, qT)
        hsT = pbig.tile([P, 8, S], BF16, tag="big")
        ln_transpose(hs, hsT, kT)

        # ---- V projection.  bv is injected into the PSUM accumulation via
        # a K=1 ones-row matmul so the evict is a plain ScalarE copy (a DVE
        # evict would queue behind the hs-LN stats and stall V on psum slots)
        for c in range(8):
            pm = pps_m.tile([P, 1024], F32, tag="pm")
            for n2 in range(2):
                nc.tensor.matmul(pm[:, n2 * 512:(n2 + 1) * 512], ones1[:],
                                 bv_sb[0:1, n2 * 512:(n2 + 1) * 512],
                                 start=True, stop=False)
                for kk in range(8):
                    nc.tensor.matmul(
                        pm[:, n2 * 512:(n2 + 1) * 512],
                        ctxT[:, kk, c * P:(c + 1) * P],
                        wv_t[kk][:, n2 * 512:(n2 + 1) * 512],
                        start=False, stop=(kk == 7))
            nc.scalar.copy(vt[:, c, :, 0:HD],
                           pm[:].rearrange("p (h c) -> p h c", c=HD))

        # ---- K/Q projections interleaved with attention.
        pT_tiles = {}

        def scores_head(h, c):
            # one head's [128, 1024] score chunk: 2 matmuls (the pair's
            # other head runs in the opposite PE row-group, so adjacent
            # A/B matmuls overlap in the array), one Exp evict.
            r0 = (h % 2) * HD
            hc = h // 2
            pm = pps_m.tile([P, 1024], F32, tag="pm", name=f"pm{h}_{c}")
            for n2 in range(2):
                nc.tensor.matmul(
                    pm[:, n2 * 512:(n2 + 1) * 512],
                    kT[r0:r0 + HD, hc, c * P:(c + 1) * P],
                    qT[r0:r0 + HD, hc, n2 * 512:(n2 + 1) * 512],
                    start=True, stop=True)
            nc.scalar.activation(pT_tiles[h][:, c, :], pm[:], AF.Exp,
                                 scale=0.125)

        def pv_m2(h, m):
            # two token-chunks (m, m+1) share one PV accumulator tile and
            # one batched reciprocal: half the DVE instruction count
            pT = pT_tiles[h]
            po = pps_o.tile([P, 2, HD + 1], F32, tag="po")
            for mi in range(2):
                for c in range(8):
                    nc.tensor.matmul(
                        po[:, mi, :],
                        pT[:, c, (m + mi) * P:(m + mi + 1) * P],
                        vt[:, c, h, :],
                        start=(c == 0), stop=(c == 7))
            rs = pstat.tile([P, 2], F32, tag="rs2")
            nc.vector.reciprocal(rs[:], po[:, :, HD])
            for mi in range(2):
                nc.vector.tensor_scalar(
                    attn[:, h // 2, m + mi, (h % 2) * HD:(h % 2) * HD + HD],
                    po[:, mi, 0:HD], rs[:, mi:mi + 1], None, ALU.mult)

        # attn^T aliases kT: chunk j of kT is dead once pair j's score
        # quads issue, and pair j's attn rows are transposed two pairs
        # later (pairs 0-5 in-stream, pairs 6-7 in the adapter tail).
        attn_T = kT[:]

        def tp_attn(j, m, pt):
            # one [128,128] transpose of pair j's attn block for token
            # chunk m into a held psum quad (evicted every 4th)
            nc.tensor.transpose(pt[:, (m % 4) * P:(m % 4 + 1) * P],
                                attn[:, j, m, :], idb[:])

        def tp_evict(j, m4, pt):
            nc.vector.tensor_copy(
                attn_T[:, j, m4 * 512:(m4 + 1) * 512].rearrange(
                    "p (mm c) -> p mm c", c=P),
                pt.rearrange("p (mm c) -> p mm c", c=P))

        # prologue: first K/Q chunk
        for n2 in range(2):
            proj_piece(wk_t, ctxT, kT, bk_sb, 0, n2)
        for n2 in range(2):
            proj_piece(wq_t, hsT, qT, bq_sb, 0, n2)
        for m8 in range(8):
            # next iteration's K/Q chunks, spread through the score stream
            # so ScalarE's exp pipeline is never starved by them
            pieces = []
            if m8 + 1 < 8:
                for wt, st_, dt_, bs in ((wk_t, ctxT, kT, bk_sb),
                                         (wq_t, hsT, qT, bq_sb)):
                    for n2 in range(2):
                        pieces.append((wt, st_, dt_, bs, m8 + 1, n2))
            hA, hB = 2 * m8, 2 * m8 + 1
            pT_tiles[hA] = ppt.tile([P, 8, S], BF16, tag="pT",
                                    name=f"pT{hA}")
            pT_tiles[hB] = ppt.tile([P, 8, S], BF16, tag="pT",
                                    name=f"pT{hB}")
            tpq = None
            for c in range(8):
                # score quads first: their psum WAR stalls at the queue
                # head, but the exp then fires right after the quad
                # instead of trailing the whole slot's work; pv, attn
                # transposes and proj pieces fill the exp window behind
                scores_head(hA, c)
                scores_head(hB, c)
                if m8 > 0:
                    pv_m2(hA - 2 if c % 2 == 0 else hB - 2, (c // 2) * 2)
                if m8 >= 2:
                    if c % 4 == 0:
                        tpq = pps_t.tile([P, 512], F32, tag="pt")
                        tpq = tpq[:].bitcast(BF16)[:, 0:512]
                    tp_attn(m8 - 2, c, tpq)
                    if c % 4 == 3:
                        tp_evict(m8 - 2, c // 4, tpq)
                if c % 2 == 1 and pieces:
                    proj_piece(*pieces.pop(0))

        # ---- adapter + residual (pairs 6-7: PV and attn^T transposes fold
        # into the m loop; pairs 0-5 were transposed during the stream).
        # The second token-half's PV/transposes interleave into the first
        # half's up-projection loop so the two n2 chains pipeline.
        def adapter_pre(n2):
            for mi in range(0, 4, 2):
                pv_m2(NH - 2, n2 * 4 + mi)
                pv_m2(NH - 1, n2 * 4 + mi)
            for j67 in (6, 7):
                pt32 = pps_t.tile([P, 512], F32, tag="pt")
                ptb = pt32[:].bitcast(BF16)[:, 0:512]
                for mi in range(4):
                    m = n2 * 4 + mi
                    nc.tensor.transpose(ptb[:, mi * P:(mi + 1) * P],
                                        attn[:, j67, m, :], idb[:])
                nc.scalar.copy(
                    attn_T[:, j67, n2 * 512:(n2 + 1) * 512].rearrange(
                        "p (mm c) -> p mm c", c=P),
                    ptb.rearrange("p (mm c) -> p mm c", c=P))

        def adapter_down(n2):
            pd = pps_m.tile([P, 1024], F32, tag="pm")
            for kk in range(8):
                nc.tensor.matmul(pd[0:HD, 0:512], wd_sb[:, kk, :],
                                 attn_T[:, kk, n2 * 512:(n2 + 1) * 512],
                                 start=(kk == 0), stop=(kk == 7))
            nc.scalar.activation(gt[0:HD, n2 * 512:(n2 + 1) * 512],
                                 pd[0:HD, 0:512], AF.Gelu_apprx_tanh,
                                 bias=bd_sb[:])

        def adapter_up_m(n2, mi):
            m = n2 * 4 + mi
            pu = pps_m.tile([P, 1024], F32, tag="pm")
            for nH in range(2):
                nc.tensor.matmul(pu[:, nH * 512:(nH + 1) * 512],
                                 gt[:, m * P:(m + 1) * P],
                                 wub_sb[:, nH * 512:(nH + 1) * 512],
                                 start=True, stop=True)
                ot = pout.tile([P, 512], F32, tag="out")
                nc.vector.tensor_tensor(
                    ot[:].rearrange("p (j dd) -> p j dd", dd=P),
                    pu[:, nH * 512:(nH + 1) * 512].rearrange(
                        "p (j dd) -> p j dd", dd=P),
                    attn[:, nH * 4:(nH + 1) * 4, m, :],
                    ALU.add)
                nc.sync.dma_start(
                    out[m * P:(m + 1) * P, nH * 512:(nH + 1) * 512],
                    ot[:])

        adapter_pre(0)
        adapter_down(0)
        adapter_pre(1)
        for mi in range(4):
            adapter_up_m(0, mi)
        adapter_down(1)
        for mi in range(4):
            adapter_up_m(1, mi)

    nc.compile()
    return nc


def make_in_maps(hidden_states, context, Wq, bq, Wk, bk, Wv, bv,
                 q_gamma, q_beta, c_gamma, c_beta, Wd, bd, Wu, bu):
    f32 = np.float32
    bf = ml_dtypes.bfloat16
    # fold LN gamma/beta into the projection weights (host-side)
    wq_e = (q_gamma[:, None] * Wq).astype(bf)
    bq_e = (bq + q_beta @ Wq).astype(f32)
    wk_e = (c_gamma[:, None] * Wk).astype(bf)
    bk_e = (bk + c_beta @ Wk).astype(f32)
    wv_e = (c_gamma[:, None] * Wv).astype(bf)
    bv_e = (bv + c_beta @ Wv).astype(f32)

    bq_r = np.ascontiguousarray(bq_e.reshape(8, P).T)   # [P, 8]
    bk_r = np.ascontiguousarray(bk_e.reshape(8, P).T)
    bv_r = bv_e.reshape(1, H).astype(bf)
    wd_b = Wd.astype(bf)
    bd_r = bd.reshape(HD, 1).astype(f32)
    wub = np.vstack([Wu, bu.reshape(1, H)]).astype(bf)

    shared = {
        "wq": np.ascontiguousarray(wq_e), "wk": np.ascontiguousarray(wk_e),
        "wv": np.ascontiguousarray(wv_e),
        "bq": bq_r, "bk": bk_r, "bv": bv_r,
        "wd": wd_b, "bd": bd_r, "wub": wub,
    }
    in_maps = []
    for b_ in range(NCORES):
        m = dict(shared)
        m["hs"] = np.ascontiguousarray(hidden_states[b_]).astype(f32)
        m["cx"] = np.ascontiguousarray(context[b_]).astype(f32)
        in_maps.append(m)
    return in_maps


_CACHE = {}


def get_program(reps=1):
    if reps not in _CACHE:
        _CACHE[reps] = build_program(reps=reps)
    return _CACHE[reps]


def kernel(**inputs):
    nc = get_program()
    in_maps = make_in_maps(**{k: np.asarray(v) for k, v in inputs.items()})
    res = run_bass_kernel_spmd(nc, in_maps, list(range(NCORES)))
    out = np.stack([res.results[c]["out"] for c in range(NCORES)], axis=0)
    return out.astype(np.float32)

